# revision 1
# baseline (speedup 1.0000x reference)
"""BERT self-attention (B=8, S=1024, D=1024, H=16, DH=64) on 8 Trainium2 cores.

Strategy: pure data-parallel over batch - each of the 8 cores runs the full
self-attention for one batch element. No collectives.

Per-core kernel layout (S=seq, D=model, H=heads, DH=64):
  - X^T built once via PE transposes (fp32, 64 tiles of 128x128).
  - Q^T[j,s], K^T[j,s] computed directly in transposed orientation
    (contraction over d_in on partitions); biases folded in as K=1 rank-1
    matmuls (b x ones).  Each weight tile is double-pumped over both 512-col
    halves of a [128,1024] PSUM tile (consecutive same-weight matmuls skip
    the serial weight reload - measured 2.2x faster).
  - V[s,j] in natural orientation (lhsT = X^T as weights), stored bf16 in a
    head-interleaved layout of 65-column blocks: [64 V cols | ones col] per
    head.  The ones column makes the context matmul emit the softmax
    denominator for free.
  - scores computed TRANSPOSED: S^T[k,q], so the attention mask (indexed by
    k) is a per-partition bias folded with the 1/sqrt(DH) scale into the Exp
    activation: P^T = exp(scale*S^T + mask[k]), output bf16.
  - context: ctx[q,0:64] + rowsum at col 64 via lhsT=P^T tile (bf16),
    rhs = V' block [128,65]; normalize with vector reciprocal +
    per-partition tensor_scalar multiply, DMA straight to DRAM.
  - attention is software-pipelined by one head: PE runs ctx(h-1) while ACT
    runs exp(h), keeping both engines busy.
  - matmul dtypes: float32r for projections/scores; bf16 for probs@V.

Built on bacc.Bacc: its compile() legalizes sync waits (1 wait/instruction
hardware limit) via move_matmul_waits_to_ldweights + generate_event_semaphores.
"""

import numpy as np

import concourse.bass as bass
import concourse.bacc as bacc
import concourse.mybir as mybir
import concourse.tile as tile
from concourse.bass_utils import run_bass_kernel_spmd
from concourse.masks import make_identity

F32 = mybir.dt.float32
F32R = mybir.dt.float32r
BF16 = mybir.dt.bfloat16

B, S, D, H = 8, 1024, 1024, 16
DH = D // H  # 64
P = 128
NT = S // P  # 8 tiles along any 1024 dim
SC = S // 512  # 2 chunks of 512
SCALE = 1.0 / float(np.sqrt(DH))
N_CORES = 8
VW = DH + 1  # 65: V block width per head (64 cols + ones col)

PHASES = 7  # bitmask: 1=x^T, 2=projections, 4=attention (profiling aid)


def emit_body(nc, dram, pools):
    (x_d, m_d, wq_d, bq_d, wk_d, bk_d, wv_d, bv_d, o_d) = dram
    (cst, xT_pool, qT_pool, kT_pool, v_pool, wx_pool, p_pool, small_pool,
     ps_t, ps_big, ps_ctx, ident) = pools

    # ---- per-body constants (mask / bias rows) ----
    mask_cols = cst.tile([P, NT], F32, name="mask_cols", tag="mask_cols")
    nc.sync.dma_start(out=mask_cols, in_=m_d.ap().rearrange("(g p) -> p g", p=P))
    ones_f32 = cst.tile([1, 512], F32, name="ones_f32", tag="ones_f32")
    nc.vector.memset(ones_f32, 1.0)
    ones_row = cst.tile([1, 512], F32R, name="ones_row", tag="ones_row")
    nc.vector.tensor_copy(ones_row, ones_f32)
    b_rows = {}
    for nm, hd in (("bq", bq_d), ("bk", bk_d), ("bv", bv_d)):
        t = cst.tile([1, D], F32R, name=f"brow_{nm}", tag=f"brow_{nm}")
        nc.sync.dma_start(out=t, in_=hd.ap().unsqueeze(0).bitcast(F32R))
        b_rows[nm] = t

    if not PHASES & 1:
        return
    # ---- phase 1: X^T via PE transposes ----
    xT = []
    for it in range(NT):
        xT.append(xT_pool.tile([P, S], F32R, name=f"xT{it}", tag=f"xT{it}"))
    for st in range(NT):
        x_t = wx_pool.tile([P, D], F32, name="x_tile", tag="wx")
        nc.sync.dma_start(out=x_t, in_=x_d.ap()[st * P : (st + 1) * P, :])
        for it in range(NT):
            pt = ps_t.tile([P, P], F32, name="pt", tag="mm")
            nc.tensor.transpose(pt, x_t[:, it * P : (it + 1) * P], ident)
            nc.vector.tensor_copy(xT[it][:, st * P : (st + 1) * P], pt)

    if not PHASES & 2:
        fin = small_pool.tile([P, DH], F32, name="fin1", tag="bounce")
        nc.vector.tensor_copy(fin, xT[0][:, 0:DH].bitcast(F32))
        nc.sync.dma_start(out=o_d.ap()[0:P, 0:DH], in_=fin)
        return

    # ---- phase 2: projections (double-pumped weights) ----
    def load_w(w_d):
        tiles = []
        for it in range(NT):
            t = wx_pool.tile([P, D], F32R, name="w_tile", tag="wx")
            nc.sync.dma_start(
                out=t, in_=w_d.ap()[it * P : (it + 1) * P, :].bitcast(F32R)
            )
            tiles.append(t)
        return tiles

    # Q^T and K^T: out[j, s] = sum_i W[i, j] * X^T[i, s] + b[j]
    proj_T = {}
    for nm, w_dram, dst_pool in (("bq", wq_d, qT_pool), ("bk", wk_d, kT_pool)):
        w_tiles = load_w(w_dram)
        dst = []
        for jt in range(NT):
            dst.append(
                dst_pool.tile([P, S], F32R, name=f"{nm}T{jt}", tag=f"{nm}T{jt}")
            )
        for jt in range(NT):
            mm = ps_big.tile([P, S], F32, name="mm", tag="big")
            for it in range(NT):
                for sc in range(SC):
                    nc.tensor.matmul(
                        mm[:, sc * 512 : (sc + 1) * 512],
                        lhsT=w_tiles[it][:, jt * P : (jt + 1) * P],
                        rhs=xT[it][:, sc * 512 : (sc + 1) * 512],
                        start=(it == 0),
                        stop=False,
                    )
            for sc in range(SC):
                nc.tensor.matmul(
                    mm[:, sc * 512 : (sc + 1) * 512],
                    lhsT=b_rows[nm][0:1, jt * P : (jt + 1) * P],
                    rhs=ones_row,
                    start=False,
                    stop=True,
                )
            nc.vector.tensor_copy(dst[jt], mm)
        proj_T[nm] = dst
    qT, kT = proj_T["bq"], proj_T["bk"]

    # V: out[s, j] = sum_i X^T[i, s] * Wv[i, j] + bv[j], stored bf16 in
    # 65-wide head blocks with a trailing ones column.
    wv_tiles = load_w(wv_d)
    v_sb = []
    for st in range(NT):
        v = v_pool.tile([P, H * VW], BF16, name=f"v{st}", tag=f"v{st}")
        nc.gpsimd.memset(v, 1.0)  # ones columns survive at h*65+64
        v_sb.append(v)
    for st in range(NT):
        mm = ps_big.tile([P, S], F32, name="mmv", tag="big")
        for it in range(NT):
            for jc in range(SC):
                nc.tensor.matmul(
                    mm[:, jc * 512 : (jc + 1) * 512],
                    lhsT=xT[it][:, st * P : (st + 1) * P],
                    rhs=wv_tiles[it][:, jc * 512 : (jc + 1) * 512],
                    start=(it == 0),
                    stop=False,
                )
        for jc in range(SC):
            nc.tensor.matmul(
                mm[:, jc * 512 : (jc + 1) * 512],
                lhsT=ones_row[0:1, 0:P],
                rhs=b_rows["bv"][0:1, jc * 512 : (jc + 1) * 512],
                start=False,
                stop=True,
            )
        dst = v_sb[st].rearrange("p (g c) -> p g c", c=VW)[:, :, 0:DH]
        src = mm.rearrange("p (g c) -> p g c", c=DH)
        nc.vector.tensor_copy(dst, src)

    if not PHASES & 4:
        fin = small_pool.tile([P, DH], F32, name="fin2", tag="bounce")
        nc.vector.tensor_copy(fin, qT[0][:, 0:DH].bitcast(F32))
        nc.sync.dma_start(out=o_d.ap()[0:P, 0:DH], in_=fin)
        fin2 = small_pool.tile([P, DH], F32, name="fin3", tag="bounce")
        nc.vector.tensor_copy(fin2, kT[0][:, 0:DH].bitcast(F32))
        nc.sync.dma_start(out=o_d.ap()[0:P, DH : 2 * DH], in_=fin2)
        return

    # ---- phase 3: attention, software-pipelined by one head ----
    def emit_scores_exp(h):
        jt, ro = h // 2, (h % 2) * DH
        pT = []
        for kt in range(NT):
            sps = ps_big.tile([P, S], F32, name="sps", tag="big")
            for qc in range(SC):
                nc.tensor.matmul(
                    sps[:, qc * 512 : (qc + 1) * 512],
                    lhsT=kT[jt][ro : ro + DH, kt * P : (kt + 1) * P],
                    rhs=qT[jt][ro : ro + DH, qc * 512 : (qc + 1) * 512],
                    start=True,
                    stop=True,
                )
            pt = p_pool.tile([P, S], BF16, name="pT", tag="pT")
            nc.scalar.activation(
                pt,
                sps,
                mybir.ActivationFunctionType.Exp,
                bias=mask_cols[:, kt : kt + 1],
                scale=SCALE,
            )
            pT.append(pt)
        return pT

    def emit_ctx(h, pT):
        for qt in range(NT):
            cps = ps_ctx.tile([P, VW], F32, name="cps", tag="ctx")
            for kt in range(NT):
                nc.tensor.matmul(
                    cps,
                    lhsT=pT[kt][:, qt * P : (qt + 1) * P],
                    rhs=v_sb[kt][:, h * VW : (h + 1) * VW],
                    start=(kt == 0),
                    stop=(kt == NT - 1),
                )
            r = small_pool.tile([P, 1], F32, name="recip", tag="recip")
            nc.vector.reciprocal(r, cps[:, DH : DH + 1])
            bounce = small_pool.tile([P, DH], F32, name="bounce", tag="bounce")
            nc.vector.tensor_scalar_mul(bounce, cps[:, 0:DH], r)
            nc.sync.dma_start(
                out=o_d.ap()[qt * P : (qt + 1) * P, h * DH : (h + 1) * DH],
                in_=bounce,
            )

    prev = None
    for h in range(H):
        pT = emit_scores_exp(h)
        if prev is not None:
            emit_ctx(h - 1, prev)
        prev = pT
    emit_ctx(H - 1, prev)


def build_program(n_reps: int = 1, n_loop: int = 0) -> bass.Bass:
    nc = bacc.Bacc(trn_type="TRN2", target_bir_lowering=False, debug=False)

    x_d = nc.declare_dram_parameter("hidden_states", [S, D], F32, isOutput=False)
    m_d = nc.declare_dram_parameter("attention_mask", [S], F32, isOutput=False)
    wq_d = nc.declare_dram_parameter("Wq", [D, D], F32, isOutput=False)
    bq_d = nc.declare_dram_parameter("bq", [D], F32, isOutput=False)
    wk_d = nc.declare_dram_parameter("Wk", [D, D], F32, isOutput=False)
    bk_d = nc.declare_dram_parameter("bk", [D], F32, isOutput=False)
    wv_d = nc.declare_dram_parameter("Wv", [D, D], F32, isOutput=False)
    bv_d = nc.declare_dram_parameter("bv", [D], F32, isOutput=False)
    o_d = nc.declare_dram_parameter("out", [S, D], F32, isOutput=True)
    dram = (x_d, m_d, wq_d, bq_d, wk_d, bk_d, wv_d, bv_d, o_d)

    with tile.TileContext(nc) as tc:
        with (
            tc.tile_pool(name="consts", bufs=1) as cst,
            tc.tile_pool(name="xT", bufs=1) as xT_pool,
            tc.tile_pool(name="qT", bufs=1) as qT_pool,
            tc.tile_pool(name="kT", bufs=1) as kT_pool,
            tc.tile_pool(name="vsb", bufs=1) as v_pool,
            tc.tile_pool(name="wx", bufs=8) as wx_pool,
            tc.tile_pool(name="pT", bufs=16) as p_pool,
            tc.tile_pool(name="small", bufs=16) as small_pool,
            # PSUM: transposes 2x1 banks, proj/scores [128,1024] 2x2 banks,
            # ctx 2x1 banks -> 8 banks total.
            tc.tile_pool(name="pst", bufs=2, space="PSUM") as ps_t,
            tc.tile_pool(name="psbig", bufs=2, space="PSUM") as ps_big,
            tc.tile_pool(name="psctx", bufs=2, space="PSUM") as ps_ctx,  # ctxT [65,512] 1 bank x2
        ):
            ident = cst.tile([P, P], F32, name="ident", tag="ident")
            make_identity(nc, ident)
            pools = (cst, xT_pool, qT_pool, kT_pool, v_pool, wx_pool, p_pool,
                     small_pool, ps_t, ps_big, ps_ctx, ident)
            if n_loop:
                with tc.For_i(0, n_loop, 1):
                    emit_body(nc, dram, pools)
            else:
                for _ in range(n_reps):
                    emit_body(nc, dram, pools)
    nc.compile()
    return nc


_NC_CACHE = None


def _get_nc():
    global _NC_CACHE
    if _NC_CACHE is None:
        _NC_CACHE = build_program()
    return _NC_CACHE


def make_in_maps(hidden_states, attention_mask, Wq, bq, Wk, bk, Wv, bv):
    hs = np.ascontiguousarray(np.asarray(hidden_states, dtype=np.float32))
    am = np.ascontiguousarray(
        np.asarray(attention_mask, dtype=np.float32).reshape(B, S)
    )
    shared = {
        "Wq": np.ascontiguousarray(np.asarray(Wq, dtype=np.float32)),
        "bq": np.ascontiguousarray(np.asarray(bq, dtype=np.float32)),
        "Wk": np.ascontiguousarray(np.asarray(Wk, dtype=np.float32)),
        "bk": np.ascontiguousarray(np.asarray(bk, dtype=np.float32)),
        "Wv": np.ascontiguousarray(np.asarray(Wv, dtype=np.float32)),
        "bv": np.ascontiguousarray(np.asarray(bv, dtype=np.float32)),
    }
    return [
        {"hidden_states": hs[b], "attention_mask": am[b], **shared}
        for b in range(B)
    ]


def kernel(hidden_states, attention_mask, Wq, bq, Wk, bk, Wv, bv):
    nc = _get_nc()
    in_maps = make_in_maps(hidden_states, attention_mask, Wq, bq, Wk, bk, Wv, bv)
    res = run_bass_kernel_spmd(nc, in_maps, list(range(N_CORES))).results
    out = np.stack([np.asarray(res[b]["out"], dtype=np.float32) for b in range(B)])
    return out



# revision 4
# speedup vs baseline: 1.2943x; 1.2943x over previous
"""BERT self-attention (B=8, S=1024, D=1024, H=16, DH=64) on 8 Trainium2 cores.

Strategy: pure data-parallel over batch - each of the 8 cores runs the full
self-attention for one batch element. No collectives.

v2 design (vs the 393us v1): everything upstream of PSUM runs in bf16
(SWDGE cast-DMAs on all loads), the context matmul is SWAPPED so V is the
stationary operand (v1 reloaded PE weights 1024x from P^T tiles; ~110ns
LDWEIGHTS per 27ns matmul), and projections/attention are software-pipelined
per head-PAIR so the ACT engine's irreducible exp stream (~147us for the
16.8M softmax elements) overlaps projection matmuls instead of serializing
after them.

Per-core layout:
  - X^T via 64 PE transposes (bf16, 1 cyc/row) from cast-loaded X tiles.
  - W col-blocks [1024,128] loaded per head-pair as [128, 8*128] bf16 tiles
    (SWDGE cast + AP rearrange); Q^T/K^T computed in [128,512] PSUM chains,
    bias folded into the DVE PSUM->SBUF copy as a per-partition
    tensor_scalar_add (no rank-1 bias matmuls).
  - scores TRANSPOSED per head pair: S^T[k,q]; the two heads of a pair sit
    on partitions 0:64 / 64:128 of qT/kT, so their K=64 matmuls auto-derive
    tile_position (0,0)/(64,0) and run CONCURRENTLY in disjoint row-groups
    of the PE array (2x on the half-array scores).
  - P^T = exp(scale*S^T + mask[k]) on ACT, bf16 out; mask is a
    per-partition bias column, so arbitrary masks are free.
  - context: ctx^T[j,q] accumulated in PSUM [65,512] with lhsT = V block
    [128 x 64 V cols | ones col] (ones col emits the softmax denominator),
    rhs = P^T streaming. 8 LDWEIGHTS of 65 cols per head instead of 64 of
    128 cols. ctx^T is copied to SBUF bf16, transposed back to natural
    [q, 65] via cheap N=65 PE transposes, normalized with DVE reciprocal +
    per-partition tensor_scalar_mul into an SBUF staging tile, and stored
    with 16 big [128,512] DMAs (first half mid-pipeline, rest at drain).
  - steady-state block for pair p: scores(p) + exp(p) + ctx(p-1) +
    Q/K projections(p+1) interleaved at 4-matmul granularity so the PE
    always has ready work while ACT paces the softmax.

Built on bacc.Bacc: its compile() legalizes sync waits (1 wait/instruction
hardware limit) via move_matmul_waits_to_ldweights + generate_event_semaphores.
"""

import numpy as np

import concourse.bass as bass
import concourse.bacc as bacc
import concourse.mybir as mybir
import concourse.tile as tile
from concourse.bass_utils import run_bass_kernel_spmd
from concourse.masks import make_identity

F32 = mybir.dt.float32
BF16 = mybir.dt.bfloat16

B, S, D, H = 8, 1024, 1024, 16
DH = D // H  # 64
P = 128
NT = S // P  # 8 tiles along any 1024 dim
NP = H // 2  # 8 head pairs
SCALE = 1.0 / float(np.sqrt(DH))
N_CORES = 8
VW = DH + 1  # 65: V block width per head (64 cols + ones col)


class Ctx:
    """Emission context: nc + dram handles + pools + persistent tiles."""

    pass


def _emit_consts(c):
    nc = c.nc
    c.mask_cols = c.cst.tile([P, NT], F32, name="mask_cols", tag="mask_cols")
    nc.sync.dma_start(out=c.mask_cols, in_=c.m_d.ap().rearrange("(g p) -> p g", p=P))
    c.bq_cols = c.cst.tile([P, NT], F32, name="bq_cols", tag="bq_cols")
    nc.sync.dma_start(out=c.bq_cols, in_=c.bq_d.ap().rearrange("(g p) -> p g", p=P))
    c.bk_cols = c.cst.tile([P, NT], F32, name="bk_cols", tag="bk_cols")
    nc.sync.dma_start(out=c.bk_cols, in_=c.bk_d.ap().rearrange("(g p) -> p g", p=P))
    c.bv_row = c.cst.tile([1, D], BF16, name="bv_row", tag="bv_row")
    nc.gpsimd.dma_start(out=c.bv_row, in_=c.bv_d.ap().unsqueeze(0))
    c.ones_row = c.cst.tile([1, P], BF16, name="ones_row", tag="ones_row")
    nc.vector.memset(c.ones_row, 1.0)
    # v_sb memsets: ones columns at h*65+64 survive the V copy.
    for st in range(NT):
        nc.vector.memset(c.v_sb[st], 1.0)


def _emit_w_dma(c, p):
    """Load Wq/Wk column-block p as [128, 8*128] bf16 tiles (cast DMA).

    tile[i%128, (i//128)*128 + j] = W[i, p*128 + j], so the it-th 128-col
    slice is the lhsT [i_part, j_cols] for contraction row-block it.
    """
    nc = c.nc
    for nm, w_d in (("q", c.wq_d), ("k", c.wk_d)):
        t = c.wqk_pool.tile([P, D], BF16, name=f"w{nm}{p}", tag="wqk")
        nc.gpsimd.dma_start(
            out=t.rearrange("p (it j) -> p it j", j=P),
            in_=w_d.ap()[:, p * P : (p + 1) * P].rearrange("(it p) j -> p it j", p=P),
        )
        c.w_tiles[(nm, p)] = t


def _emit_wv_dma(c):
    nc = c.nc
    c.wv_tiles = []
    for it in range(NT):
        t = c.stage_pool.tile([P, D], BF16, name=f"wv{it}", tag="stage")
        nc.gpsimd.dma_start(out=t, in_=c.wv_d.ap()[it * P : (it + 1) * P, :])
        c.wv_tiles.append(t)


def _emit_phase_a(c):
    """Cast-load X and build X^T (bf16) via PE transposes."""
    nc = c.nc
    xbs = []
    for st in range(NT):
        xb = c.stage_pool.tile([P, D], BF16, name=f"xb{st}", tag="stage")
        nc.gpsimd.dma_start(out=xb, in_=c.x_d.ap()[st * P : (st + 1) * P, :])
        xbs.append(xb)
    # W col-blocks for pairs 0/1 queue behind the X loads on the SWDGE ring.
    _emit_w_dma(c, 0)
    _emit_w_dma(c, 1)
    for st in range(NT):
        for it in range(NT):
            pt = c.ps_tr.tile([P, P], BF16, name="pt", tag="tr")
            nc.tensor.transpose(pt, xbs[st][:, it * P : (it + 1) * P], c.ident)
            nc.vector.tensor_copy(c.xT[it][:, st * P : (st + 1) * P], pt)


def _gen_qk_proj(c, p):
    """8 groups: Q^T[p], K^T[p] in four [128,512] PSUM chains of 8 matmuls."""
    nc = c.nc
    dst_q = c.qkT_pool.tile([P, S], BF16, name=f"qT{p}", tag="qT")
    dst_k = c.qkT_pool.tile([P, S], BF16, name=f"kT{p}", tag="kT")
    c.qT[p], c.kT[p] = dst_q, dst_k
    groups = []
    for nm, dst, bcol in (("q", dst_q, c.bq_cols), ("k", dst_k, c.bk_cols)):
        for sc in range(2):
            for half in range(2):
                def g(nm=nm, dst=dst, bcol=bcol, sc=sc, half=half):
                    w = c.w_tiles[(nm, p)]
                    if half == 0:
                        ps = c.ps_proj.tile([P, 512], F32, name="psp", tag="proj")
                        c._proj_ps = ps
                    else:
                        ps = c._proj_ps
                    for it in range(4 * half, 4 * half + 4):
                        nc.tensor.matmul(
                            ps,
                            lhsT=w[:, it * P : (it + 1) * P],
                            rhs=c.xT[it][:, sc * 512 : (sc + 1) * 512],
                            start=(it == 0),
                            stop=(it == NT - 1),
                        )
                    if half == 1:
                        nc.vector.tensor_scalar_add(
                            dst[:, sc * 512 : (sc + 1) * 512], ps, bcol[:, p : p + 1]
                        )
                groups.append(g)
    return groups


def _emit_v_proj(c):
    """V natural [s, j] into 65-wide head blocks, bf16, + bias matmul."""
    nc = c.nc
    for st in range(NT):
        for jc in range(2):
            ps = c.ps_proj.tile([P, 512], F32, name="psv", tag="proj")
            for it in range(NT):
                nc.tensor.matmul(
                    ps,
                    lhsT=c.xT[it][:, st * P : (st + 1) * P],
                    rhs=c.wv_tiles[it][:, jc * 512 : (jc + 1) * 512],
                    start=(it == 0),
                    stop=False,
                )
            nc.tensor.matmul(
                ps,
                lhsT=c.ones_row[0:1, 0:P],
                rhs=c.bv_row[0:1, jc * 512 : (jc + 1) * 512],
                start=False,
                stop=True,
            )
            dst = c.v_sb[st].rearrange("p (g c) -> p g c", c=VW)[
                :, jc * 8 : (jc + 1) * 8, 0:DH
            ]
            nc.vector.tensor_copy(dst, ps.rearrange("p (g c) -> p g c", c=DH))


def _gen_scores(c, p):
    """8 groups (one per kt): 4 score matmuls (2 heads row-tiled) + 2 exps."""
    nc = c.nc
    groups = []
    for kt in range(NT):
        def g(kt=kt):
            for hl, ro in ((0, 0), (1, DH)):
                ps = c.ps_sc.tile([P, S], F32, name="pss", tag="sc")
                for qc in range(2):
                    nc.tensor.matmul(
                        ps[:, qc * 512 : (qc + 1) * 512],
                        lhsT=c.kT[p][ro : ro + DH, kt * P : (kt + 1) * P],
                        rhs=c.qT[p][ro : ro + DH, qc * 512 : (qc + 1) * 512],
                        start=True,
                        stop=True,
                    )
                pt = c.pT_pool.tile([P, S], BF16, name="pT", tag="pT")
                nc.scalar.activation(
                    pt,
                    ps,
                    mybir.ActivationFunctionType.Exp,
                    bias=c.mask_cols[:, kt : kt + 1],
                    scale=SCALE,
                )
                c.pT[(p, hl, kt)] = pt
        groups.append(g)
    return groups


def _gen_ctx(c, p):
    """8 groups: ctx^T chunks (head, qc) of pair p -> transpose -> normalize.

    chunk c_ = s//2 over (hl, qc); group s%2 emits 4 of its 8 accumulating
    matmuls; the closing group adds copy + 4 transposes + normalize.
    """
    nc = c.nc
    groups = []
    for s in range(8):
        def g(s=s):
            c_, half = divmod(s, 2)
            hl, qc = divmod(c_, 2)
            h = 2 * p + hl
            if half == 0:
                ps = c.ps_ctx.tile([VW, 512], F32, name="psc", tag="ctx")
                c._ctx_ps = ps
            else:
                ps = c._ctx_ps
            for kt in range(4 * half, 4 * half + 4):
                nc.tensor.matmul(
                    ps,
                    lhsT=c.v_sb[kt][:, h * VW : (h + 1) * VW],
                    rhs=c.pT[(p, hl, kt)][:, qc * 512 : (qc + 1) * 512],
                    start=(kt == 0),
                    stop=(kt == NT - 1),
                )
            if half == 1:
                csb = c.ctxsb_pool.tile([VW, 512], BF16, name="csb", tag="ctxsb")
                nc.vector.tensor_copy(csb, ps)
                for l in range(4):
                    qt = qc * 4 + l
                    trp = c.ps_tr.tile([P, VW], BF16, name="trp", tag="tr")
                    nc.tensor.transpose(
                        trp, csb[:, l * P : (l + 1) * P], c.ident[0:VW, 0:VW]
                    )
                    r = c.small.tile([P, 1], F32, name="r", tag="r")
                    nc.vector.reciprocal(r, trp[:, DH : DH + 1])
                    nc.vector.tensor_scalar_mul(
                        c.out_sb[qt][:, h * DH : (h + 1) * DH], trp[:, 0:DH], r
                    )
        groups.append(g)
    return groups


def _emit_out_dma(c, half, engines):
    for qt in range(NT):
        eng = engines[qt % len(engines)]
        eng.dma_start(
            out=c.o_d.ap()[qt * P : (qt + 1) * P, half * 512 : (half + 1) * 512],
            in_=c.out_sb[qt][:, half * 512 : (half + 1) * 512],
        )


def emit_body(nc, dram, pools):
    c = Ctx()
    c.nc = nc
    (c.x_d, c.m_d, c.wq_d, c.bq_d, c.wk_d, c.bk_d, c.wv_d, c.bv_d, c.o_d) = dram
    (c.cst, c.stage_pool, c.wqk_pool, c.xT_pool, c.qkT_pool, c.v_pool,
     c.pT_pool, c.ctxsb_pool, c.out_pool, c.small,
     c.ps_sc, c.ps_proj, c.ps_ctx, c.ps_tr, c.ident) = pools

    c.w_tiles, c.qT, c.kT, c.pT = {}, {}, {}, {}
    c.xT = [
        c.xT_pool.tile([P, S], BF16, name=f"xT{it}", tag=f"xT{it}")
        for it in range(NT)
    ]
    c.v_sb = [
        c.v_pool.tile([P, H * VW], BF16, name=f"v{st}", tag=f"v{st}")
        for st in range(NT)
    ]
    c.out_sb = [
        c.out_pool.tile([P, S], F32, name=f"o{qt}", tag=f"o{qt}")
        for qt in range(NT)
    ]

    _emit_consts(c)
    # ---- preamble: X^T, QK0+sc0, QK1+sc1, V, QK2, ctx0 ----
    _emit_phase_a(c)  # also issues W dmas for pairs 0,1
    for g in _gen_qk_proj(c, 0):
        g()
    for g in _gen_scores(c, 0):
        g()
    _emit_w_dma(c, 2)
    for g in _gen_qk_proj(c, 1):
        g()
    for g in _gen_scores(c, 1):
        g()
    _emit_wv_dma(c)
    _emit_v_proj(c)
    _emit_w_dma(c, 3)
    for g in _gen_qk_proj(c, 2):
        g()
    for g in _gen_ctx(c, 0):
        g()

    # ---- steady-state blocks: p = 2..7 ----
    for p in range(2, NP):
        if p + 2 < NP:
            _emit_w_dma(c, p + 2)
        if p == 5:
            # heads 0..7 (pairs 0..3, ctx done in block 4) cover out cols 0:512
            _emit_out_dma(c, 0, [nc.sync])
        sc_g = _gen_scores(c, p)
        ctx_g = _gen_ctx(c, p - 1)
        proj_g = _gen_qk_proj(c, p + 1) if p + 1 < NP else [None] * 8
        for s in range(8):
            sc_g[s]()
            ctx_g[s]()
            if proj_g[s] is not None:
                proj_g[s]()

    # ---- drain: ctx7 + remaining output ----
    for g in _gen_ctx(c, NP - 1):
        g()
    _emit_out_dma(c, 1, [nc.sync, nc.scalar])


def build_program(n_reps: int = 1, n_loop: int = 0) -> bass.Bass:
    nc = bacc.Bacc(trn_type="TRN2", target_bir_lowering=False, debug=False)

    x_d = nc.declare_dram_parameter("hidden_states", [S, D], F32, isOutput=False)
    m_d = nc.declare_dram_parameter("attention_mask", [S], F32, isOutput=False)
    wq_d = nc.declare_dram_parameter("Wq", [D, D], F32, isOutput=False)
    bq_d = nc.declare_dram_parameter("bq", [D], F32, isOutput=False)
    wk_d = nc.declare_dram_parameter("Wk", [D, D], F32, isOutput=False)
    bk_d = nc.declare_dram_parameter("bk", [D], F32, isOutput=False)
    wv_d = nc.declare_dram_parameter("Wv", [D, D], F32, isOutput=False)
    bv_d = nc.declare_dram_parameter("bv", [D], F32, isOutput=False)
    o_d = nc.declare_dram_parameter("out", [S, D], F32, isOutput=True)
    dram = (x_d, m_d, wq_d, bq_d, wk_d, bk_d, wv_d, bv_d, o_d)

    with tile.TileContext(nc) as tc:
        with (
            tc.tile_pool(name="consts", bufs=1) as cst,
            tc.tile_pool(name="stage", bufs=8) as stage_pool,  # X then Wv tiles
            tc.tile_pool(name="wqk", bufs=4) as wqk_pool,
            tc.tile_pool(name="xT", bufs=1) as xT_pool,
            tc.tile_pool(name="qkT", bufs=3) as qkT_pool,
            tc.tile_pool(name="vsb", bufs=1) as v_pool,
            tc.tile_pool(name="pT", bufs=32) as pT_pool,
            tc.tile_pool(name="ctxsb", bufs=2) as ctxsb_pool,
            tc.tile_pool(name="osb", bufs=1) as out_pool,
            tc.tile_pool(name="small", bufs=16) as small_pool,
            # PSUM is bank-granular (8 banks x 2KB): sc 2x2 + proj 1 +
            # ctx 1 + tr 2x1 = 8 banks
            tc.tile_pool(name="pssc", bufs=2, space="PSUM") as ps_sc,
            tc.tile_pool(name="psproj", bufs=1, space="PSUM") as ps_proj,
            tc.tile_pool(name="psctx", bufs=1, space="PSUM") as ps_ctx,
            tc.tile_pool(name="pstr", bufs=2, space="PSUM") as ps_tr,
        ):
            ident = cst.tile([P, P], BF16, name="ident", tag="ident")
            make_identity(nc, ident)
            pools = (cst, stage_pool, wqk_pool, xT_pool, qkT_pool, v_pool,
                     pT_pool, ctxsb_pool, out_pool, small_pool,
                     ps_sc, ps_proj, ps_ctx, ps_tr, ident)
            if n_loop:
                with tc.For_i(0, n_loop, 1):
                    emit_body(nc, dram, pools)
            else:
                for _ in range(n_reps):
                    emit_body(nc, dram, pools)
    nc.compile()
    return nc


_NC_CACHE = None


def _get_nc():
    global _NC_CACHE
    if _NC_CACHE is None:
        _NC_CACHE = build_program()
    return _NC_CACHE


def make_in_maps(hidden_states, attention_mask, Wq, bq, Wk, bk, Wv, bv):
    hs = np.ascontiguousarray(np.asarray(hidden_states, dtype=np.float32))
    am = np.ascontiguousarray(
        np.asarray(attention_mask, dtype=np.float32).reshape(B, S)
    )
    shared = {
        "Wq": np.ascontiguousarray(np.asarray(Wq, dtype=np.float32)),
        "bq": np.ascontiguousarray(np.asarray(bq, dtype=np.float32)),
        "Wk": np.ascontiguousarray(np.asarray(Wk, dtype=np.float32)),
        "bk": np.ascontiguousarray(np.asarray(bk, dtype=np.float32)),
        "Wv": np.ascontiguousarray(np.asarray(Wv, dtype=np.float32)),
        "bv": np.ascontiguousarray(np.asarray(bv, dtype=np.float32)),
    }
    return [
        {"hidden_states": hs[b], "attention_mask": am[b], **shared}
        for b in range(B)
    ]


def kernel(hidden_states, attention_mask, Wq, bq, Wk, bk, Wv, bv):
    nc = _get_nc()
    in_maps = make_in_maps(hidden_states, attention_mask, Wq, bq, Wk, bk, Wv, bv)
    res = run_bass_kernel_spmd(nc, in_maps, list(range(N_CORES))).results
    out = np.stack([np.asarray(res[b]["out"], dtype=np.float32) for b in range(B)])
    return out
